# revision 11
# baseline (speedup 1.0000x reference)
"""Trainium2 Bass kernel for nn_DenseExpert (MoE dense-expert gated blend).

Math (full problem, B=8192, E=8, U=512, D=512):
    h[b,e,u] = sum_d x[b,d] * alpha[e,u,d]
    r[b,u]   = sum_e g[b,e] * h[b,e,u] + sum_e g[b,e] * beta[e,u]

Strategy:
  - Data-parallel over batch B across 8 NeuronCores (1024 rows each);
    alpha/beta replicated. No collectives.
  - Host-side layout prep only (transposes/concat): x^T, alpha^T, and
    gb = [g^T | beta] so the contraction dim d lands on SBUF partitions
    with contiguous DMA lines and so tensors consumed together arrive
    in one DMA (the fused fp32r matmul instruction has a single
    semaphore-wait slot, so each matmul may depend on at most one
    outstanding transfer).
  - Per core: for each expert e, h_e = x @ alpha[e]^T as 4 accumulating
    fp32r matmuls per 128-row batch tile (fp32r at N=512 runs at bf16
    rate, ~78 TF/s). The gated blend is a single fused DVE op per
    (expert, tile): acc = psum * g[:,e] + acc (scalar_tensor_tensor).
    The bias sum_e g[b,e]*beta[e,u] is one K=8 matmul per tile, folded
    into expert 0's blend.
  - A throwaway matmul into a scratch PSUM bank "touches" each expert's
    alpha tile so the real matmuls never need both a DMA wait and a
    PSUM-reuse wait on the same instruction.
"""

import numpy as np
from contextlib import ExitStack

try:
    import concourse.bass as bass
except ImportError:  # fallback if concourse isn't on the default path
    import sys

    sys.path.insert(0, "/opt/trn_rl_repo")
    import concourse.bass as bass
from concourse import bacc

import concourse.mybir as mybir
import concourse.tile as tile
from concourse.bass_utils import run_bass_kernel_spmd

B, E, U, D = 8192, 8, 512, 512
N_CORES = 8
BC = B // N_CORES  # 1024 batch rows per core
P = 128
M_TILES = BC // P  # 8 batch tiles per core
K_TILES = D // P  # 4 contraction chunks
F32 = mybir.dt.float32
F32R = mybir.dt.float32r

_NC_CACHE = {}
last_results = None  # BassKernelResults of the most recent run (for test harness)


def _build_nc():
    nc = bacc.Bacc("TRN2", target_bir_lowering=False, debug=False)

    xT = nc.dram_tensor("xT", [D, BC], F32R, kind="ExternalInput").ap()
    g = nc.dram_tensor("g", [BC, E], F32, kind="ExternalInput").ap()
    # gb = [g^T | beta] packed so one DMA covers both bias-matmul operands
    gb = nc.dram_tensor("gb", [E, BC + U], F32R, kind="ExternalInput").ap()
    alphaT = nc.dram_tensor("alphaT", [E, D, U], F32R, kind="ExternalInput").ap()
    out = nc.dram_tensor("out", [BC, U], F32, kind="ExternalOutput").ap()

    mult = mybir.AluOpType.mult
    add = mybir.AluOpType.add

    with tile.TileContext(nc) as tc, ExitStack() as ctx:
        xt_pool = ctx.enter_context(tc.tile_pool(name="xt", bufs=K_TILES))
        sml_pool = ctx.enter_context(tc.tile_pool(name="sml", bufs=1))
        at_pool = ctx.enter_context(tc.tile_pool(name="at", bufs=E))
        acc_pool = ctx.enter_context(tc.tile_pool(name="acc", bufs=M_TILES))
        pe_pool = ctx.enter_context(tc.tile_pool(name="pe", bufs=4, space="PSUM"))
        pb_pool = ctx.enter_context(tc.tile_pool(name="pb", bufs=2, space="PSUM"))

        # ---- preload x^T (one DMA per k-chunk), g, gb ----
        xts = []
        for k in range(K_TILES):
            xt_t = xt_pool.tile([P, BC], F32R, tag="xt", name=f"xt{k}")
            nc.sync.dma_start(xt_t[:], xT[k * P : (k + 1) * P, :])
            xts.append(xt_t)

        gsb = []
        for m in range(M_TILES):
            g_t = sml_pool.tile([P, E], F32, tag=f"g{m}", name=f"g{m}")
            nc.sync.dma_start(g_t[:], g[m * P : (m + 1) * P, :])
            gsb.append(g_t)

        gb_t = sml_pool.tile([E, BC + U], F32R, tag="gb", name="gb")
        nc.sync.dma_start(gb_t[:], gb[:, :])

        # ---- alpha^T: ONE DMA per expert into a [P, K_TILES, U] tile ----
        ats = []
        for e in range(E):
            a_t = at_pool.tile([P, K_TILES, U], F32R, tag="at", name=f"at{e}")
            nc.sync.dma_start(
                a_t[:], alphaT[e].rearrange("(k p) u -> p k u", p=P)
            )
            ats.append(a_t)

        # ---- bias: b_m = g @ beta (K=8 matmuls, copied to SBUF) ----
        biases = []
        for m in range(M_TILES):
            pb_t = pb_pool.tile([P, U], F32, tag="pb", name=f"pb{m}")
            nc.tensor.matmul(
                pb_t[:],
                gb_t[:, m * P : (m + 1) * P],
                gb_t[:, BC : BC + U],
                start=True,
                stop=True,
            )
            b_t = acc_pool.tile([P, U], F32, tag="bias", name=f"bias{m}")
            nc.vector.tensor_copy(b_t[:], pb_t[:])
            biases.append(b_t)

        # ---- experts ----
        accs = [None] * M_TILES
        for e in range(E):
            for m in range(M_TILES):
                ms = bass.ts(m, P)
                pe_t = pe_pool.tile([P, U], F32, tag="pe", name=f"pe{e}_{m}")
                for k in range(K_TILES):
                    nc.tensor.matmul(
                        pe_t[:],
                        xts[k][:, ms],
                        ats[e][:, k, :],
                        start=(k == 0),
                        stop=(k == K_TILES - 1),
                    )
                if e == 0:
                    acc_t = acc_pool.tile([P, U], F32, tag="acc", name=f"acc{m}")
                    # acc = h_0 * g[:,0] + bias
                    nc.vector.scalar_tensor_tensor(
                        acc_t[:], pe_t[:], gsb[m][:, 0:1], biases[m][:],
                        op0=mult, op1=add,
                    )
                    accs[m] = acc_t
                else:
                    # acc += h_e * g[:,e]
                    nc.vector.scalar_tensor_tensor(
                        accs[m][:], pe_t[:], gsb[m][:, e : e + 1], accs[m][:],
                        op0=mult, op1=add,
                    )

        # ---- write out ----
        for m in range(M_TILES):
            nc.sync.dma_start(out[m * P : (m + 1) * P, :], accs[m][:])

    nc.compile()
    return nc


def _get_nc():
    if "nc" not in _NC_CACHE:
        _NC_CACHE["nc"] = _build_nc()
    return _NC_CACHE["nc"]


def kernel(x, g, alpha, beta, _trace=False, _trace_kwargs=None):
    global last_results
    x = np.ascontiguousarray(np.asarray(x, dtype=np.float32))
    g = np.ascontiguousarray(np.asarray(g, dtype=np.float32))
    alpha = np.asarray(alpha, dtype=np.float32)
    beta = np.ascontiguousarray(np.asarray(beta, dtype=np.float32))

    alphaT = np.ascontiguousarray(alpha.transpose(0, 2, 1))  # [E, D, U]

    in_maps = []
    for c in range(N_CORES):
        sl = slice(c * BC, (c + 1) * BC)
        gc = g[sl]
        in_maps.append(
            {
                "xT": np.ascontiguousarray(x[sl].T),  # [D, BC]
                "g": gc,  # [BC, E]
                "gb": np.ascontiguousarray(
                    np.concatenate([gc.T, beta], axis=1)
                ),  # [E, BC + U]
                "alphaT": alphaT,
            }
        )

    nc = _get_nc()
    res = run_bass_kernel_spmd(
        nc,
        in_maps,
        list(range(N_CORES)),
        trace=_trace,
        **(_trace_kwargs or {}),
    )
    last_results = res
    return np.concatenate([r["out"] for r in res.results], axis=0)


# revision 12
# speedup vs baseline: 1.1395x; 1.1395x over previous
"""Trainium2 Bass kernel for nn_DenseExpert (MoE dense-expert gated blend).

Math (full problem, B=8192, E=8, U=512, D=512):
    h[b,e,u] = sum_d x[b,d] * alpha[e,u,d]
    r[b,u]   = sum_e g[b,e] * h[b,e,u] + sum_e g[b,e] * beta[e,u]

Strategy:
  - Data-parallel over batch B across 8 NeuronCores (1024 rows each);
    alpha/beta replicated. No collectives.
  - With all 8 cores streaming a replicated 8 MB alpha, the kernel is
    paced by aggregate HBM bandwidth, so the matmul operands (x, alpha)
    are converted to bf16 on the host — this halves the dominant
    traffic and keeps the PE at full rate (1 cycle/row). Measured
    scale-relative error ~1e-3. The small bias matmul stays fp32r.
  - Host-side layout prep only (transposes/concat/casts): x^T, alpha^T,
    gb = [g^T | beta], so the contraction dim d lands on SBUF
    partitions with contiguous DMA lines.
  - Per core: for each expert e, h_e = x @ alpha[e]^T as 4 accumulating
    bf16 matmuls per 128-row batch tile, looped k-outer across all 8
    batch tiles (8 PSUM banks) so compute can start as soon as the
    first [128,512] alpha slice lands. The gated blend is one fused DVE
    op per (expert, tile): acc = psum * g[:,e] + acc. The bias
    sum_e g[b,e]*beta[e,u] is one K=8 matmul per tile, folded into
    expert 0's blend.
  - DMA issue order is a priority schedule: gb/g first (bias matmuls
    warm the PE), then xt/alpha[0] k-slices interleaved, then the
    remaining experts.
"""

import numpy as np
from contextlib import ExitStack

try:
    import concourse.bass as bass
except ImportError:  # fallback if concourse isn't on the default path
    import sys

    sys.path.insert(0, "/opt/trn_rl_repo")
    import concourse.bass as bass
from concourse import bacc

import concourse.mybir as mybir
import concourse.tile as tile
from concourse.bass_utils import run_bass_kernel_spmd

B, E, U, D = 8192, 8, 512, 512
N_CORES = 8
BC = B // N_CORES  # 1024 batch rows per core
P = 128
M_TILES = BC // P  # 8 batch tiles per core
K_TILES = D // P  # 4 contraction chunks
F32 = mybir.dt.float32
F32R = mybir.dt.float32r
BF16 = mybir.dt.bfloat16

_NC_CACHE = {}
last_results = None  # BassKernelResults of the most recent run (for test harness)


def _build_nc():
    nc = bacc.Bacc("TRN2", target_bir_lowering=False, debug=False)

    xT = nc.dram_tensor("xT", [D, BC], BF16, kind="ExternalInput").ap()
    g = nc.dram_tensor("g", [BC, E], F32, kind="ExternalInput").ap()
    # gb = [g^T | beta] packed so one DMA covers both bias-matmul operands
    gb = nc.dram_tensor("gb", [E, BC + U], F32R, kind="ExternalInput").ap()
    alphaT = nc.dram_tensor("alphaT", [E, D, U], BF16, kind="ExternalInput").ap()
    out = nc.dram_tensor("out", [BC, U], F32, kind="ExternalOutput").ap()

    mult = mybir.AluOpType.mult
    add = mybir.AluOpType.add

    with tile.TileContext(nc) as tc, ExitStack() as ctx:
        xt_pool = ctx.enter_context(tc.tile_pool(name="xt", bufs=K_TILES))
        sml_pool = ctx.enter_context(tc.tile_pool(name="sml", bufs=1))
        at_pool = ctx.enter_context(tc.tile_pool(name="at", bufs=E * K_TILES))
        acc_pool = ctx.enter_context(tc.tile_pool(name="acc", bufs=M_TILES))
        ps_pool = ctx.enter_context(tc.tile_pool(name="ps", bufs=8, space="PSUM"))

        # ---- DMA priority order: bias operands, g, then xt/alpha[0]
        # interleaved per k-chunk, then experts 1.. ----
        gb_t = sml_pool.tile([E, BC + U], F32R, tag="gb", name="gb")
        nc.sync.dma_start(gb_t[:], gb[:, :])

        gsb = []
        for m in range(M_TILES):
            g_t = sml_pool.tile([P, E], F32, tag=f"g{m}", name=f"g{m}")
            nc.sync.dma_start(g_t[:], g[m * P : (m + 1) * P, :])
            gsb.append(g_t)

        xts = [None] * K_TILES
        at = {}

        def load_xt(k):
            xt_t = xt_pool.tile([P, BC], BF16, tag="xt", name=f"xt{k}")
            nc.sync.dma_start(xt_t[:], xT[k * P : (k + 1) * P, :])
            xts[k] = xt_t

        def load_at(e, k):
            a_t = at_pool.tile([P, U], BF16, tag="at", name=f"at{e}_{k}")
            nc.sync.dma_start(a_t[:], alphaT[e, k * P : (k + 1) * P, :])
            at[(e, k)] = a_t

        for k in range(K_TILES):
            load_xt(k)
            load_at(0, k)
        for e in range(1, E):
            for k in range(K_TILES):
                load_at(e, k)

        # ---- bias: b_m = g @ beta (K=8 fp32r matmuls, copied to SBUF) ----
        biases = []
        for m in range(M_TILES):
            pb_t = ps_pool.tile([P, U], F32, tag="ps", name=f"pb{m}")
            nc.tensor.matmul(
                pb_t[:],
                gb_t[:, m * P : (m + 1) * P],
                gb_t[:, BC : BC + U],
                start=True,
                stop=True,
            )
            b_t = acc_pool.tile([P, U], F32, tag="bias", name=f"bias{m}")
            nc.vector.tensor_copy(b_t[:], pb_t[:])
            biases.append(b_t)

        # ---- experts: k-outer so compute starts on the first alpha slice ----
        accs = [None] * M_TILES
        for e in range(E):
            pes = []
            for m in range(M_TILES):
                pes.append(ps_pool.tile([P, U], F32, tag="ps", name=f"pe{e}_{m}"))
            for k in range(K_TILES):
                for m in range(M_TILES):
                    nc.tensor.matmul(
                        pes[m][:],
                        xts[k][:, bass.ts(m, P)],
                        at[(e, k)][:],
                        start=(k == 0),
                        stop=(k == K_TILES - 1),
                    )
            for m in range(M_TILES):
                if e == 0:
                    acc_t = acc_pool.tile([P, U], F32, tag="acc", name=f"acc{m}")
                    # acc = h_0 * g[:,0] + bias
                    nc.vector.scalar_tensor_tensor(
                        acc_t[:], pes[m][:], gsb[m][:, 0:1], biases[m][:],
                        op0=mult, op1=add,
                    )
                    accs[m] = acc_t
                else:
                    # acc += h_e * g[:,e]
                    nc.vector.scalar_tensor_tensor(
                        accs[m][:], pes[m][:], gsb[m][:, e : e + 1], accs[m][:],
                        op0=mult, op1=add,
                    )

        # ---- write out ----
        for m in range(M_TILES):
            nc.sync.dma_start(out[m * P : (m + 1) * P, :], accs[m][:])

    nc.compile()
    return nc


def _get_nc():
    if "nc" not in _NC_CACHE:
        _NC_CACHE["nc"] = _build_nc()
    return _NC_CACHE["nc"]


def kernel(x, g, alpha, beta, _trace=False, _trace_kwargs=None):
    global last_results
    import ml_dtypes

    bf16 = ml_dtypes.bfloat16
    x = np.asarray(x, dtype=np.float32)
    g = np.ascontiguousarray(np.asarray(g, dtype=np.float32))
    alpha = np.asarray(alpha, dtype=np.float32)
    beta = np.ascontiguousarray(np.asarray(beta, dtype=np.float32))

    # [E, D, U] in bf16 for halved DMA traffic
    alphaT = np.ascontiguousarray(alpha.transpose(0, 2, 1).astype(bf16))
    xTb = np.ascontiguousarray(x.T.astype(bf16))  # [D, B]

    in_maps = []
    for c in range(N_CORES):
        sl = slice(c * BC, (c + 1) * BC)
        gc = g[sl]
        in_maps.append(
            {
                "xT": np.ascontiguousarray(xTb[:, sl]),  # [D, BC] bf16
                "g": gc,  # [BC, E] f32
                "gb": np.ascontiguousarray(
                    np.concatenate([gc.T, beta], axis=1)
                ),  # [E, BC + U] f32
                "alphaT": alphaT,  # [E, D, U] bf16 (replicated)
            }
        )

    nc = _get_nc()
    res = run_bass_kernel_spmd(
        nc,
        in_maps,
        list(range(N_CORES)),
        trace=_trace,
        **(_trace_kwargs or {}),
    )
    last_results = res
    return np.concatenate([r["out"] for r in res.results], axis=0)
